# revision 25
# baseline (speedup 1.0000x reference)
"""MRU encoding kernel for Trainium2 (8 NeuronCores, batch-parallel).

Problem (B=32, T=2048, D=300):
    z = tanh(x @ Wz.T + bz); o = tanh(x @ Wo.T + bo)
    c_t = g_t*c_{t-1} + (1-g_t)*z_t   (c_{-1}=0, scan over T)
    out = o * c

Per-core (4 batch rows) layout is [channel, time]:
  - host pre-transposes x,g to [b, D, T]; x gets a ones-row (301) so the
    bias rides in the matmul contraction; weights are fed as [D+1, 256] =
    [W.T; b] for e-columns 0..255, and the ragged e-columns 256..299 of
    BOTH weights are merged into one [D+1, 128] tensor (Wz at cols 0..43,
    -Wo at cols 64..107) so one matmul pass computes z-ragged AND o-ragged
    for a batch row: 15 PE passes per row instead of 18.
  - o is produced NEGATED via tanh(scale=-1) (j-tiles) or negated weights
    (ragged): with bneg=(g-1)*z = -(1-g)z the hardware scan
    state=g*state+bneg yields -c, and (-o)*(-c) = o*c.
  - engine balance (TimelineSim: PE 54.5us, DVE 53.4, ACT 43, DMA 42,
    Pool 38): gm1=g-1 runs on Pool (depends only on the g load, so it is
    hoisted clear of every chain); bneg=gm1*z, the scan, and the final
    multiply run on DVE; tanh on ACT. The Pool engine only accepts
    TensorScalar/TensorTensor/Memset/Copy opcodes on real HW.
  - the ragged chain is PER ROW, shuffle-free: one [0:108]-partition tanh
    gives [z@0..43 | -o@64..107] in one tile; the scan then writes its
    output PARTITION-SHIFTED to 64..107 so the final multiply pairs it
    with -o alignment-free (engines allow shifted outputs, and shifted
    activation inputs, at 64-partition offsets).
  - PSUM is 8 half-T tiles (2 banks each, bufs=4): each half frees after
    ONE tanh, so the next tile's matmuls never wait a full-tile drain.
    All z-passes run before all o-passes inside a tile: z psum frees
    early and the o passes start after the deferred wo load lands.
  - PE p-state warmup: ~3us of dummy matmuls on junk SBUF before the
    first x chunk arrives, so the real stream runs at full clock; a dummy
    tanh absorbs the one-time activation-table load.
  - stores are queued and ISSUE-DELAYED by FLUSH_K completed chains: a
    store on the in-order ACT/SP rings must never sit ahead of a tanh
    waiting for chain data. Last-pair stores ride the (by then idle) SP
    ring.
  - kernel head: the first chain runs gm1+bneg as DVE halves (Pool's 3us
    gm1 would gate it) with the g plane loaded in 512-col-first chunks;
    kernel tail: the last pair runs ragged FIRST and the final chain is
    HALF-split (TTS=2; quarters add ACT-init overhead right where the
    in-order ACT ring is the drain bottleneck) with its bneg on Pool.
  - the kernel END is DVE-bound, not PE-bound: DVE (scans + bneg + final
    muls, ~53us) runs ~100% busy from mid-kernel to ~8us past PE-end, so
    the close is tuned for DVE: chain 6 (the last pair's first ragged)
    runs its final multiply on Pool (PMC -- the ONE spot where a 4.3us
    Pool TT mul fits without head-of-line-blocking later gm1s on Pool's
    in-order queue), the tail chain issues both chained scans
    back-to-back on DVE before its multiplies (TSM/TMS=2), and xp=3
    prefetch removes the pair-boundary PE gaps.
  - NOTE (from op-legality probes via neuronxcc): scalar_tensor_tensor
    is DVE-only -- walrus' neuron_isa_check rejects it on Pool, even
    though TimelineSim accepts it. A Pool-stt variant of this kernel
    simmed at 63.7us but cannot compile for real hardware.
"""

import numpy as np

import concourse.bass as bass
import concourse.mybir as mybir
import concourse.tile as tile
from concourse import bacc
from concourse.bass_utils import run_bass_kernel_spmd

B, T, D = 32, 2048, 300
NCORES = 8
BC = B // NCORES  # 4 batch rows per core
DP = D + 1  # ones-row at index 300 carries the bias
WJ = 256  # e-columns covered by the two full j-tiles
TS = 512  # moving-operand max free dim
NT = T // TS
F32 = mybir.dt.float32
F32R = mybir.dt.float32r
F16 = mybir.dt.float16

KC = [(0, 128), (128, 128), (256, 45)]  # k-chunks (incl. ones row)

CFG = {"mm16": True, "plane16": True, "c16": True, "out16": True}
N_WARM = 7  # dummy 512-col matmuls before the first real pass
FLUSH_K = 4  # store issue delay, in completed chains
ORDER0 = "jjRjj"  # pair-0 schedule (R=ragged, j=next j-tile)
ORDER1 = "Rjjjj"  # last-pair schedule

_CACHE: dict = {}
_KNOBS: dict = {}


def kernel_knob(name, default):
    return _KNOBS.get(name, default)


def _build_program(reps=1, bufs=None, cfg=None):
    c = dict(CFG)
    if cfg:
        c.update(cfg)
    mm_dt = F16 if c["mm16"] else F32R
    pl_dt = F16 if c["plane16"] else F32
    c_dt = F16 if c["c16"] else F32
    out_dt = F16 if c["out16"] else F32

    # xp=3 prefetches the next pair's x a full row earlier (kills the
    # ~1.4us of pair-boundary PE gaps); zp=3 loosens z/o recycling
    bf = {"xp": 3, "gp": 2, "zp": 3, "ep": 4, "ps": 4}
    if bufs:
        bf.update(bufs)

    nc = bacc.Bacc("TRN2", target_bir_lowering=False, debug=False, num_devices=NCORES)

    d_x = nc.dram_tensor("xt", [BC, DP, T], mm_dt, kind="ExternalInput").ap()
    d_g = nc.dram_tensor("gt", [BC, D, T], pl_dt, kind="ExternalInput").ap()
    d_wz = nc.dram_tensor("wz", [DP, WJ], mm_dt, kind="ExternalInput").ap()
    d_wo = nc.dram_tensor("wo", [DP, WJ], mm_dt, kind="ExternalInput").ap()
    d_wr = nc.dram_tensor("wr", [DP, 128], mm_dt, kind="ExternalInput").ap()
    # replicas share ONE output tensor: keeps the PJRT buffer count (and its
    # per-call overhead) constant across reps so marginal timing is clean
    d_out0 = nc.dram_tensor("outt", [BC, D, T], out_dt, kind="ExternalOutput").ap()
    d_outs = [d_out0] * reps

    with tile.TileContext(nc) as tc:
        with (
            tc.tile_pool(name="wp", bufs=1) as wp,
            tc.tile_pool(name="xp", bufs=bf["xp"]) as xp,
            tc.tile_pool(name="gp", bufs=bf["gp"]) as gp,
            tc.tile_pool(name="zp", bufs=bf["zp"]) as zp,
            tc.tile_pool(name="ep", bufs=bf["ep"]) as ep,
            tc.tile_pool(name="ps", bufs=bf["ps"], space="PSUM") as ps,
        ):
            wts = {}
            for nm, w in (("wz", WJ), ("wo", WJ), ("wr", 128)):
                wts[nm] = wp.tile([128, 3, w], mm_dt, tag=nm, name=f"w_{nm}")

            def load_w(nm, dram, eng):
                t = wts[nm]
                eng.dma_start(
                    t[:, 0:2, :], dram[0:256, :].rearrange("(c p) m -> p c m", c=2)
                )
                eng.dma_start(t[:45, 2, :], dram[256:DP, :])

            # wz rides the ACT ring (needed first); wo/wr are issued later on
            # the SP ring so the shared HWDGE serves the first x loads sooner
            load_w("wz", d_wz, nc.scalar)

            pending = []  # (dram_ap, sbuf_ap, ring) store queue, delayed
            store_eng = [nc.scalar]  # ring for newly queued stores
            marks = []  # pending-length marks, one per completed chain
            chix = [0]  # chain ordinal: chains whose ordinal is in the PMC
            # knob run their final multiply on Pool (TT is Pool-legal) --
            # DVE is the tail-region bottleneck, Pool idles ~45%

            def chain_done():
                marks.append(len(pending))

            def flush_stores(keep_chains):
                """Issue all queued stores except those of the most recent
                `keep_chains` chains."""
                if len(marks) < keep_chains:
                    return
                upto = marks[-keep_chains] if keep_chains else len(pending)
                for _ in range(upto):
                    ds, rs, eng = pending.pop(0)
                    eng.dma_start(ds, rs)
                marks[:] = [m - upto for m in marks]

            def bneg_split(bneg, gs, z_ap, mj, bneg_pool=False,
                           first=False, pre_gm1=None):
                """bneg=(g-1)z: gm1=g-1 runs on Pool (TensorScalar is one
                of the few opcodes the Pool engine accepts) and depends only
                on the g load, so the scheduler hoists it WAY before the
                chain; the multiply is a single 2x-mode DVE op -- except for
                the kernel-tail chain, where it runs as two half-T Pool
                multiplies to keep the final scans unblocked on DVE.
                pre_gm1 supplies a pair-start hoisted gm1 tile, keeping
                Pool's in-order queue clear of gm1s when PMC parks a
                scan-gated multiply there."""
                if pre_gm1 is not None and not first:
                    gm1 = pre_gm1
                    if bneg_pool:
                        nq = kernel_knob("BQ", 2)
                        Q = T // nq
                        for q in range(nq):
                            lo, hi = q * Q, (q + 1) * Q
                            nc.gpsimd.tensor_mul(
                                bneg[:mj, lo:hi], gm1[:mj, lo:hi],
                                z_ap[:, lo:hi]
                            )
                    else:
                        nc.vector.tensor_mul(
                            bneg[:mj, :], gm1[:mj, :], z_ap[:, :]
                        )
                    return
                gm1 = ep.tile([128, T], pl_dt, tag="gm1", name="gm1_t")
                if first:
                    # kernel head: DVE is idle and Pool's 3us gm1 would gate
                    # the very first chain -- run gm1+bneg as DVE quarters
                    # that track the 512-chunked g arrival
                    nfq = kernel_knob("FQ", 2)
                    Q = T // nfq
                    for q in range(nfq):
                        lo, hi = q * Q, (q + 1) * Q
                        nc.vector.tensor_scalar_add(
                            gm1[:mj, lo:hi], gs[:, lo:hi], -1.0
                        )
                        nc.vector.tensor_mul(
                            bneg[:mj, lo:hi], gm1[:mj, lo:hi], z_ap[:, lo:hi]
                        )
                    return
                nc.gpsimd.tensor_scalar_add(gm1[:mj, :], gs[:, :], -1.0)
                if bneg_pool:
                    nq = kernel_knob("BQ", 2)
                    Q = T // nq
                    for q in range(nq):
                        lo, hi = q * Q, (q + 1) * Q
                        nc.gpsimd.tensor_mul(
                            bneg[:mj, lo:hi], gm1[:mj, lo:hi], z_ap[:, lo:hi]
                        )
                else:
                    nc.vector.tensor_mul(
                        bneg[:mj, :], gm1[:mj, :], z_ap[:, :]
                    )

            def elemwise(gs, z_ap, oneg_ap, mj, stores, tsplit=2,
                         mul_pool=False, bneg_pool=False, first=False,
                         pre_gm1=None, tail_sm=False):
                """bneg=(g-1)z split across Pool+DVE -> scan(-c) on DVE ->
                out=(-o)*(-c) on DVE; stores is a list of
                (res_slice, dram_slice). The scan/mul run in T-halves (scan
                chained via `initial`) so each half starts as soon as its
                tanh half lands."""
                bneg = ep.tile([128, T], pl_dt, tag="bneg", name="bneg_t")
                cneg = ep.tile([128, T], c_dt, tag="c", name="cneg_t")
                res = ep.tile([128, T], out_dt, tag="res", name="res_t")
                bneg_split(bneg, gs, z_ap, mj, bneg_pool=bneg_pool,
                           first=first, pre_gm1=pre_gm1)
                tw = T // tsplit
                if tail_sm and kernel_knob("TSM", True):
                    # kernel-tail chain: chained scans issue back-to-back on
                    # DVE (no mul parked between them), then the multiplies
                    # run at TMS granularity so the LAST store is small
                    for h in range(tsplit):
                        hs = slice(h * tw, (h + 1) * tw)
                        init = (0.0 if h == 0
                                else cneg[:mj, h * tw - 1 : h * tw])
                        nc.vector.tensor_tensor_scan(
                            cneg[:mj, hs], gs[:, hs], bneg[:mj, hs], init,
                            op0=mybir.AluOpType.mult,
                            op1=mybir.AluOpType.add,
                        )
                    nms = kernel_knob("TMS", 2)
                    mw = T // nms
                    for m in range(nms):
                        ms = slice(m * mw, (m + 1) * mw)
                        nc.vector.tensor_mul(
                            res[:mj, ms], oneg_ap[:, ms], cneg[:mj, ms]
                        )
                        # the LAST store rides the (idle-by-now) ACT ring so
                        # it never queues behind earlier stores' data-waits
                        # on SP's in-order SEQ
                        eng = (nc.scalar if m == nms - 1
                               and kernel_knob("TSE", False) else store_eng[0])
                        for rs, ds in stores:
                            pending.append((ds[:, ms],
                                            res[rs[0] : rs[1], ms], eng))
                    return
                for h in range(tsplit):
                    hs = slice(h * tw, (h + 1) * tw)
                    init = 0.0 if h == 0 else cneg[:mj, h * tw - 1 : h * tw]
                    nc.vector.tensor_tensor_scan(
                        cneg[:mj, hs], gs[:, hs], bneg[:mj, hs], init,
                        op0=mybir.AluOpType.mult, op1=mybir.AluOpType.add,
                    )
                    if mul_pool:
                        # kernel tail: Pool is idle, DVE is the critical
                        # engine -- the final multiplies go to GPSIMD so the
                        # scans stream back-to-back on DVE
                        nc.gpsimd.tensor_mul(
                            res[:mj, hs], oneg_ap[:, hs], cneg[:mj, hs]
                        )
                    else:
                        nc.vector.tensor_mul(
                            res[:mj, hs], oneg_ap[:, hs], cneg[:mj, hs]
                        )
                    for rs, ds in stores:
                        # stores ride the ACT ring but are ISSUE-DELAYED by
                        # three chains (see flush_stores): by the time the
                        # in-order ACT SEQ reaches them their data is ready,
                        # so they never block the next tile's tanh
                        pending.append((ds[:, hs], res[rs[0] : rs[1], hs],
                                        store_eng[0]))

            def half_mms(pa, pb, wname, xt, msl):
                """One projection into TWO half-T psum tiles (2 banks
                each): each half releases after ONE tanh instead of two, so
                the next tile's matmuls never wait on a full-tile drain. The
                k2 (ragged-k) passes run last so the first tile never waits
                on the k2 x-chunk (it is the 5th DMA of the row)."""
                for k in (0, 1, 2):
                    kn = KC[k][1]
                    for tb in range(NT):
                        p = pa if tb < 2 else pb
                        nc.tensor.matmul(
                            p[:, bass.ts(tb % 2, TS)],
                            lhsT=wts[wname][:kn, k, msl],
                            rhs=xt[:kn, k, bass.ts(tb, TS)],
                            start=(k == 0), stop=(k == 2),
                        )

            def proj_mms(pzs, pos, xt, msl):
                """ALL z-passes run before ALL o-passes: the z psum halves
                close (and free) early, and the o matmuls start after the
                deferred wo weights arrive -- keeps PE gap-free."""
                half_mms(pzs[0], pzs[1], "wz", xt, msl)
                half_mms(pos[0], pos[1], "wo", xt, msl)

            def ragged_mms(pra, prb, xt, wname="wr"):
                """Merged ragged pass: one m=128 matmul computes z-ragged
                (psum parts 0..43) AND -o-ragged (parts 64..107, negated
                weights) per k-chunk; two half-T psum tiles as in half_mms."""
                for k in (0, 1, 2):
                    kn = KC[k][1]
                    for tb in range(NT):
                        p = pra if tb < 2 else prb
                        nc.tensor.matmul(
                            p[:, bass.ts(tb % 2, TS)],
                            lhsT=wts[wname][:kn, k, :],
                            rhs=xt[:kn, k, bass.ts(tb, TS)],
                            start=(k == 0), stop=(k == 2),
                        )

            def ragged_chain(pra, prb, g2, b, d_out, mul_pool=False):
                """Per-row shuffle-free ragged chain: zo = tanh(pr[0:108]) is
                [z@0..43 | -o@64..107] (wr's o-cols are negated); the scan
                writes cneg partition-SHIFTED to 64..107 so the final
                multiply pairs it with -o alignment-free."""
                mul_pool = mul_pool or chix[0] in kernel_knob("PMC", (6,))
                chix[0] += 1
                flush_stores(FLUSH_K)
                zo = zp.tile([128, T], pl_dt, tag="z" if b % 2 == 0 else "o",
                             name="t_zo")
                nc.scalar.activation(
                    zo[0:108, 0:1024], pra[0:108, :],
                    mybir.ActivationFunctionType.Tanh, scale=1.0,
                )
                nc.scalar.activation(
                    zo[0:108, 1024:T], prb[0:108, :],
                    mybir.ActivationFunctionType.Tanh, scale=1.0,
                )
                bnr = ep.tile([44, T], pl_dt, tag="bnr", name="bnr_t")
                cneg = ep.tile([128, T], c_dt, tag="c", name="cnr_t")
                res = ep.tile([128, T], out_dt, tag="res", name="resr_t")
                gm1r = ep.tile([44, T], pl_dt, tag="gm1r", name="gm1r_t")
                nc.gpsimd.tensor_scalar_add(gm1r[:, :], g2[:, :], -1.0)
                nc.vector.tensor_mul(bnr[:, :], gm1r[:, :], zo[0:44, :])
                nc.vector.tensor_tensor_scan(
                    cneg[64:108, :], g2[:, :], bnr[:, :], 0.0,
                    op0=mybir.AluOpType.mult, op1=mybir.AluOpType.add,
                )
                mul_eng = nc.gpsimd if mul_pool else nc.vector
                mul_eng.tensor_mul(
                    res[64:108, :], zo[64:108, :], cneg[64:108, :]
                )
                pending.append((d_out[b, 256:D, :], res[64:108, :],
                                store_eng[0]))
                chain_done()

            # PE p-state warmup: ~3us of dummy matmuls on junk SBUF keep
            # the PE "busy" from ~1us so the REAL first matmuls run at full
            # clock (the ramp needs 3us of continuous execution). The dummy
            # accumulation groups are reset by the real start=True passes.
            # A 1-element dummy tanh absorbs the one-time LoadActFuncSet
            # (1.3us) so it never sits in the psum-recycle critical path.
            junk = wp.tile([128, 516], mm_dt, tag="junk", name="junk_t")
            nc.gpsimd.memset(junk[:, :], 0.25)
            nc.scalar.activation(
                junk[0:1, 512:513], junk[0:1, 513:514],
                mybir.ActivationFunctionType.Tanh, scale=1.0,
            )
            warm_ps = ps.tile([128, 512], F32, tag="p", name="psum_warm")
            for _ in range(N_WARM):
                nc.tensor.matmul(
                    warm_ps[:, 0:512],
                    lhsT=junk[:, 0:128],
                    rhs=junk[:, 0:512],
                    start=True, stop=True,
                )

            for d_out in d_outs:
              for pair in range(BC // 2):
                b0, b1 = 2 * pair, 2 * pair + 1
                xts = {}
                gts = {}
                g2s = {}
                for b in (b0, b1):
                    xt = xp.tile([128, 3, T], mm_dt, tag="x", name="xt_t")
                    # k-major load order matches the matmul k-pass order so
                    # the first do_j streams without waiting on later chunks;
                    # the first row's k0 half is split again so the very
                    # first real matmul starts one DMA-slot earlier
                    if pair == 0 and b == b0 and kernel_knob("XS", 0):
                        nc.sync.dma_start(xt[:, 0, 0:512], d_x[b, 0:128, 0:512])
                        nc.sync.dma_start(xt[:, 0, 512:1024], d_x[b, 0:128, 512:1024])
                    else:
                        nc.sync.dma_start(xt[:, 0, 0:1024], d_x[b, 0:128, 0:1024])
                    nc.sync.dma_start(xt[:, 0, 1024:T], d_x[b, 0:128, 1024:T])
                    nc.sync.dma_start(xt[:, 1, 0:1024], d_x[b, 128:256, 0:1024])
                    nc.sync.dma_start(xt[:, 1, 1024:T], d_x[b, 128:256, 1024:T])
                    nc.sync.dma_start(xt[:45, 2, :], d_x[b, 256:DP, :])
                    xts[b] = xt
                    if pair == 0 and b == b0:
                        # deferred: wo behind row b0's x chunks (o-passes
                        # need it ~3us before the first chain needs g); wr
                        # is issued after the g loads (ragged runs ~15us in)
                        load_w("wo", d_wo, nc.sync)
                    gt = gp.tile([128, 2, T], pl_dt, tag="g", name="gt_t")
                    if pair == 0 and b == b0:
                        # 512-first chunks: the first chain starts its DVE
                        # work the moment the first g quarter lands
                        for lo, hi in ((0, 1024), (1024, T)):
                            nc.sync.dma_start(
                                gt[:, :, lo:hi],
                                d_g[b, 0:256, lo:hi].rearrange(
                                    "(c p) t -> p c t", c=2),
                            )
                    else:
                        nc.sync.dma_start(
                            gt[:, :, :],
                            d_g[b, 0:256, :].rearrange("(c p) t -> p c t", c=2),
                        )
                    gts[b] = gt
                    g2 = gp.tile([44, T], pl_dt, tag=f"g2{b % 2}", name="g2_t")
                    nc.sync.dma_start(g2[:, :], d_g[b, 256:D, :])
                    g2s[b] = g2
                    if pair == 0 and b == b0:
                        load_w("wr", d_wr, nc.sync)

                def do_j(b, j, tsplit=1, mul_pool=False, bneg_pool=False,
                         first=False):
                    mul_pool = mul_pool or chix[0] in kernel_knob("PMC", (6,))
                    # PBC chains run the bneg multiply on Pool (paced by the
                    # z-tanh anyway) -- each takes 1127ns off saturated DVE;
                    # requires GH hoisting so no gm1 queues behind it
                    pbc = chix[0] in kernel_knob("PBC", ())
                    chix[0] += 1
                    flush_stores(FLUSH_K)
                    m0 = 128 * j
                    pza = ps.tile([128, 1024], F32, tag="p", name="psum_za")
                    pzb = ps.tile([128, 1024], F32, tag="p", name="psum_zb")
                    poa = ps.tile([128, 1024], F32, tag="p", name="psum_oa")
                    pob = ps.tile([128, 1024], F32, tag="p", name="psum_ob")
                    proj_mms((pza, pzb), (poa, pob), xts[b],
                             slice(m0, m0 + 128))
                    z_j = zp.tile([128, T], pl_dt, tag="z", name="t_z")
                    oneg_j = zp.tile([128, T], pl_dt, tag="o", name="t_o")
                    # z tanhs first: in-order ACT must not park a ready
                    # z-half behind an o-half whose psum closes later (z frees
                    # its psum for the next tile's matmuls). The kernel-head
                    # tile quarters them so the first chain starts sooner.
                    nsp = tsplit if tsplit >= 4 else 2
                    nzq = (kernel_knob("FQ", 2) // 2) if first else 1
                    for h, p in ((0, pza), (1, pzb)):
                        for q in range(nzq):
                            qw = 1024 // nzq
                            hs = slice(h * 1024 + q * qw,
                                       h * 1024 + (q + 1) * qw)
                            nc.scalar.activation(
                                z_j[:, hs], p[:, q * qw : (q + 1) * qw],
                                mybir.ActivationFunctionType.Tanh, scale=1.0,
                            )
                    for h in range(nsp):
                        hw = T // nsp
                        hs = slice(h * hw, (h + 1) * hw)
                        p = (poa, pob)[h * hw // 1024]
                        ph = slice(h * hw % 1024, h * hw % 1024 + hw)
                        nc.scalar.activation(
                            oneg_j[:, hs], p[:, ph],
                            mybir.ActivationFunctionType.Tanh, scale=-1.0,
                        )
                    elemwise(
                        gts[b][:, j, :], z_j[:, :], oneg_j[:, :], 128,
                        [((0, 128), d_out[b, m0 : m0 + 128, :])],
                        tsplit=tsplit, mul_pool=mul_pool,
                        bneg_pool=bneg_pool or pbc, first=first,
                        pre_gm1=gm1s.get((b, j)), tail_sm=bneg_pool,
                    )
                    chain_done()

                def ragged_all(mul_pool=False):
                    pr0a = ps.tile([128, 1024], F32, tag="p", name="psum_r0a")
                    pr0b = ps.tile([128, 1024], F32, tag="p", name="psum_r0b")
                    ragged_mms(pr0a, pr0b, xts[b0], "wr")
                    ragged_chain(pr0a, pr0b, g2s[b0], b0, d_out,
                                 mul_pool=mul_pool)
                    pr1a = ps.tile([128, 1024], F32, tag="p", name="psum_r1a")
                    pr1b = ps.tile([128, 1024], F32, tag="p", name="psum_r1b")
                    ragged_mms(pr1a, pr1b, xts[b1], "wr")
                    ragged_chain(pr1a, pr1b, g2s[b1], b1, d_out,
                                 mul_pool=mul_pool)

                last = pair == BC // 2 - 1
                if last:
                    # SP ring is past all its loads: stores there stop
                    # punching descriptor-gen holes into the ACT tanh stream
                    store_eng[0] = nc.sync
                order = ORDER1 if last else ORDER0
                jseq = [(b0, 0), (b0, 1), (b1, 0), (b1, 1)]
                gm1s = {}
                gh = kernel_knob("GH", 0)
                if (gh == 2) or (gh == 1 and last):
                    # hoist ALL this pair's gm1s to the pair start: they
                    # depend only on the g loads, and emitting them first
                    # keeps Pool's in-order queue clear so PMC multiplies
                    # can't head-of-line-block a later chain's gm1
                    for hb, hj in jseq:
                        if pair == 0 and hb == b0 and hj == 0:
                            continue  # head chain builds bneg on DVE
                        hg = ep.tile([128, T], pl_dt, tag="gm1",
                                     name="gm1_t")
                        nc.gpsimd.tensor_scalar_add(
                            hg[:, :], gts[hb][:, hj, :], -1.0)
                        gm1s[(hb, hj)] = hg
                ji = 0
                for ci, ch in enumerate(order):
                    if ch == "R":
                        ragged_all()
                    else:
                        b, j = jseq[ji]
                        ji += 1
                        tail = last and ci == len(order) - 1
                        pre_tail = last and ci == len(order) - 2
                        first = pair == 0 and ci == 0
                        ts = kernel_knob("TTS", 2) if tail else (
                            kernel_knob("FQ", 2) if first else (
                                kernel_knob("PTS", 1) if pre_tail
                                else kernel_knob("MTS", 1)))
                        do_j(b, j, tsplit=ts,
                             mul_pool=pre_tail and kernel_knob("MP", False),
                             bneg_pool=tail and kernel_knob("BP", True),
                             first=first)
              flush_stores(0)

    nc.compile()
    return nc


def kernel(gate_encoding, inputs_encoding, Wz, bz, Wo, bo):
    gate_encoding = np.asarray(gate_encoding, dtype=np.float32)
    inputs_encoding = np.asarray(inputs_encoding, dtype=np.float32)
    Wz = np.asarray(Wz, dtype=np.float32)
    bz = np.asarray(bz, dtype=np.float32)
    Wo = np.asarray(Wo, dtype=np.float32)
    bo = np.asarray(bo, dtype=np.float32)

    mm_np = np.float16 if CFG["mm16"] else np.float32
    pl_np = np.float16 if CFG["plane16"] else np.float32

    def aug(Wmat, bvec):
        w = np.zeros((DP, D), dtype=np.float32)
        w[:D, :] = Wmat.T
        w[D, :] = bvec
        return w

    wz_full = aug(Wz, bz)
    wo_full = aug(Wo, bo)
    wz_in = wz_full[:, :WJ].astype(mm_np)
    wo_in = wo_full[:, :WJ].astype(mm_np)
    wr_in = np.zeros((DP, 128), dtype=np.float32)
    wr_in[:, 0:44] = wz_full[:, WJ:D]
    wr_in[:, 64:108] = -wo_full[:, WJ:D]  # negated: tanh(scale=+1) gives -o
    wr_in = wr_in.astype(mm_np)

    if "nc" not in _CACHE:
        _CACHE["nc"] = _build_program()
    nc = _CACHE["nc"]

    in_maps = []
    for cc in range(NCORES):
        xs = inputs_encoding[cc * BC : (cc + 1) * BC]  # [BC, T, D]
        gs = gate_encoding[cc * BC : (cc + 1) * BC]
        xt = np.empty((BC, DP, T), dtype=mm_np)
        xt[:, :D, :] = xs.transpose(0, 2, 1)
        xt[:, D, :] = 1.0
        gt = gs.transpose(0, 2, 1).astype(pl_np)
        in_maps.append({"xt": xt, "gt": gt, "wz": wz_in, "wo": wo_in,
                        "wr": wr_in})

    res = run_bass_kernel_spmd(nc, in_maps, core_ids=list(range(NCORES)))

    out = np.empty((B, T, D), dtype=np.float32)
    for cc in range(NCORES):
        out[cc * BC : (cc + 1) * BC] = (
            res.results[cc]["outt"].transpose(0, 2, 1).astype(np.float32)
        )
    return out

